# revision 34
# baseline (speedup 1.0000x reference)
"""Trainium2 Bass kernel for nn_CGCN (curvature-weighted GCN + pair decoder).

Strategy (8 NeuronCores, SPMD):
  - Edges sharded by DESTINATION node: core c owns nodes [c*N/8, (c+1)*N/8)
    and every edge whose col lands there (host bins/sorts; index plumbing
    only). The scatter-add stays core-local - no [N,H] all-reduce.
  - Edge weights ew = Linear(func_k(curvature)) on device via Horner in
    t = exp(-c).
  - Degrees: per-node padded layout [node-on-partition, window x Dmax],
    fp32 vector reduce (exact per-node sums, no long-prefix cancellation).
  - y = dinv * (x @ W) per 128-node window (bf16), AllGathered.
  - Aggregation: per 128-edge chunk, S[p,j] = (dst_rel[p]==j) built on DVE;
    psum[window,H] += S^T @ (y_rows * ew) on PE. y rows fetched with
    dma_gather (int16 idx), the table split in 4 segments of 2*NPAD rows so
    indices fit int16. Chunk counts are per-(window, segment) (max over the
    8 cores - SPMD shares one program); per-core shortfall is padded with
    idx=-1, which the gather ucode trims from descriptor generation, so
    padding costs no Q7 time. Gather tiles are pre-zeroed on the Scalar
    engine so trimmed (stale) slots can never inject NaN into the matmul.
  - x1 = relu(dinv*(agg+y)+b) in bf16, AllGathered (halves collective and
    decoder-gather bytes vs f32).
  - Decoder: pairs grouped by (seg(e1), seg(e2)) into 16 variable-size
    groups (chunks padded to a multiple of 4); dma_gather idx padded with
    -1 (trimmed; NaN from stale slots stays column-isolated and those
    columns are discarded by the host un-permute). feat@dec1_w decomposed
    as e1@(Wa+Wc) + e2@(Wa+Wd) + (e1*e2)@Wb, all in bf16 (F32R matmuls are
    ~4x slower on PE).
"""
import sys

for _p in ("/opt/trn_rl_repo",):
    if _p not in sys.path:
        sys.path.append(_p)

import numpy as np
from contextlib import ExitStack

import concourse.bass as bass
import concourse.tile as tile
from concourse import mybir, bacc
from concourse.masks import make_identity

F32 = mybir.dt.float32
F32R = mybir.dt.float32r
BF16 = mybir.dt.bfloat16
I32 = mybir.dt.int32
I16 = mybir.dt.int16

NCORES = 8
NSEG = 4          # table segments (2 cores' rows each) so idx fits int16
WARM_W = 0        # gather tiles are pre-zeroed instead of warmed
PAD_IDX = -1      # -1: ucode trims trailing pads (no Q7 cost); 0: plain pad


def _wrap16(a):
    """[128, n] per-slot idx array -> dma_gather idx layout [128, n*8] int16.

    Slot (chunk c, partition p) maps to idx-list position i = c*128 + p;
    the ucode reads position i from [p16=i%16, col=c*8 + (i%128)//16],
    partitions replicated x8.
    """
    p128, n = a.shape
    assert p128 == 128
    m = np.zeros((16, n, 8), np.int16)
    for s16 in range(8):
        m[:, :, s16] = a[s16 * 16:(s16 + 1) * 16, :]
    m = m.reshape(16, n * 8)
    return np.tile(m, (8, 1)).copy()


class Dims:
    def __init__(self, N, E, P, FIN, H, D1, CS_tab, D, PG_tab, M_tab, PGM):
        self.N, self.E, self.P = N, E, P
        self.FIN, self.H, self.D1 = FIN, H, D1
        self.CS_tab = CS_tab           # [W][NSEG] chunks per (window, seg)
        self.D = D                     # max node in-degree
        self.PG_tab = PG_tab           # [16] decoder chunks per pair-group
        self.M_tab = M_tab             # [W][NSEG] max-over-cores idx count
        self.PGM = PGM                 # [16] max-over-cores pair count
        self.NSH = N // NCORES
        self.W = (self.NSH + 127) // 128
        self.NPAD = self.W * 128
        self.SEGR = 2 * self.NPAD      # rows per table segment
        assert len(CS_tab) == self.W
        # chunk-column base per (w, s); layout is w-major then s
        self.colbase = np.zeros((self.W, NSEG), np.int64)
        acc = 0
        for w in range(self.W):
            for s in range(NSEG):
                self.colbase[w, s] = acc
                acc += CS_tab[w][s]
        self.NCH = acc                 # total chunk columns
        self.WS = [sum(CS_tab[w]) for w in range(self.W)]
        self.CSMAX = max(max(CS_tab[w]) for w in range(self.W))
        self.WD = self.W * D
        self.PPC = P // NCORES
        # decoder groups (chunks, multiple of 4)
        self.gbase = np.zeros(16, np.int64)
        acc = 0
        for g in range(16):
            self.gbase[g] = acc
            acc += PG_tab[g]
        self.PCH2 = acc
        self.KF = FIN // 128
        self.WH = 2 if self.W % 2 == 0 else 1
        assert self.SEGR <= 32768


def chunk_params(edge_index, idx, N):
    """Per-(window,seg) chunk counts / D / decoder group sizes (max over cores)."""
    NSH = N // NCORES
    W = (NSH + 127) // 128
    NPAD = W * 128
    SEGR = 2 * NPAD
    col = np.asarray(edge_index[1], np.int64)
    row = np.asarray(edge_index[0], np.int64)
    order = np.argsort(col, kind="stable")
    col_s, row_s = col[order], row[order]
    D = int(np.bincount(col_s, minlength=N).max())
    seg_of_row = ((row_s // NSH) * NPAD + (row_s % NSH)) // SEGR
    cs_tab = np.zeros((W, NSEG), np.int64)
    m_tab = np.zeros((W, NSEG), np.int64)
    for c in range(NCORES):
        lo, hi = np.searchsorted(col_s, [c * NSH, (c + 1) * NSH])
        wof = (col_s[lo:hi] - c * NSH) >> 7
        sof = seg_of_row[lo:hi]
        cnt = np.bincount(wof * NSEG + sof, minlength=W * NSEG).reshape(W, NSEG)
        cs_tab = np.maximum(cs_tab, (cnt + 127) // 128)
        m_tab = np.maximum(m_tab, cnt)
    cs_tab[:, 0] = np.maximum(cs_tab[:, 0], 1)   # >=1 matmul per window
    CS_tab = [[int(x) for x in cs_tab[w]] for w in range(W)]
    M_tab = [[int(x) for x in m_tab[w]] for w in range(W)]
    # decoder groups
    i1 = np.asarray(idx[0], np.int64)
    i2 = np.asarray(idx[1], np.int64)
    g1 = ((i1 // NSH) * NPAD + (i1 % NSH)) // SEGR
    g2 = ((i2 // NSH) * NPAD + (i2 % NSH)) // SEGR
    PPC = i1.shape[0] // NCORES
    pg = np.zeros(16, np.int64)
    pgm = np.zeros(16, np.int64)
    for c in range(NCORES):
        g = g1[c * PPC:(c + 1) * PPC] * NSEG + g2[c * PPC:(c + 1) * PPC]
        cnt = np.bincount(g, minlength=16)
        pg = np.maximum(pg, (cnt + 127) // 128)
        pgm = np.maximum(pgm, cnt)
    pg = np.maximum(pg, 1)
    PG_tab = [int(x) for x in pg]
    PGM = [int(x) for x in pgm]
    return CS_tab, D, PG_tab, M_tab, PGM


def preprocess(d, x, curvature, edge_index, idx, b_gcn, lin1_w, lin1_b):
    """Index plumbing: sort/bin/pad edges and pairs, build per-core inputs."""
    row = np.asarray(edge_index[0], dtype=np.int64)
    col = np.asarray(edge_index[1], dtype=np.int64)
    cur = np.asarray(curvature, dtype=np.float32)
    order = np.argsort(col, kind="stable")
    row_s, col_s, cur_s = row[order], col[order], cur[order]
    bounds = np.searchsorted(col_s, np.arange(NCORES + 1) * d.NSH)

    iota_f = np.broadcast_to(np.arange(128, dtype=np.float32), (128, 128)).copy()
    b_mat = np.broadcast_to(np.asarray(b_gcn, np.float32)[None, :], (128, d.H)).copy()
    linw_rep = np.broadcast_to(np.asarray(lin1_w, np.float32)[:, 0][None, :], (128, 10)).copy()
    linb_rep = np.full((128, 1), np.float32(np.asarray(lin1_b).reshape(-1)[0]), np.float32)

    def gidx(nodes):
        return ((nodes // d.NSH) * d.NPAD + (nodes % d.NSH)).astype(np.int64)

    maps = []
    perms = []
    for c in range(NCORES):
        lo, hi = bounds[c], bounds[c + 1]
        colr = (col_s[lo:hi] - c * d.NSH).astype(np.int64)
        grow = gidx(row_s[lo:hi])
        curc = cur_s[lo:hi]
        n_c = colr.shape[0]
        wof = colr >> 7
        sof = grow // d.SEGR
        # order edges by (window, segment) [stable within = by col]
        okey = np.argsort(wof * NSEG + sof, kind="stable")
        colr, grow, curc = colr[okey], grow[okey], curc[okey]
        wof, sof = wof[okey], sof[okey]
        # chunk column for each edge
        cnt = np.bincount(wof * NSEG + sof, minlength=d.W * NSEG).reshape(d.W, NSEG)
        start = np.zeros(d.W * NSEG, np.int64)
        start[1:] = np.cumsum(cnt.reshape(-1))[:-1]
        pos = np.arange(n_c) - start[wof * NSEG + sof]
        ci = d.colbase[wof, sof] + (pos >> 7)
        p = (pos & 127).astype(np.int64)

        dstf = np.full((128, d.NCH), -1.0, np.float32)
        dstf[p, ci] = (colr & 127).astype(np.float32)
        curb = np.zeros((128, d.NCH), np.float32)
        curb[p, ci] = curc
        # idx padding: zeros up to the shared per-call count M_tab (so the
        # static num_idxs_reg matches the ucode's trailing-negative trim on
        # every core), then -1 to chunk capacity (trimmed, no Q7 cost).
        ridx = np.full((128, d.NCH), PAD_IDX, np.int64)
        for w in range(d.W):
            for s in range(NSEG):
                cs = d.CS_tab[w][s]
                if cs == 0:
                    continue
                c0 = d.colbase[w, s]
                m = d.M_tab[w][s]
                sl = np.arange(cnt[w, s], m)
                ridx[sl & 127, c0 + (sl >> 7)] = 0
        ridx[p, ci] = grow - sof * d.SEGR
        g16 = _wrap16(ridx.astype(np.int16))

        # degree layout (per-node padded)
        colr2 = col_s[lo:hi] - c * d.NSH
        cur2 = cur_s[lo:hi]
        nstart = np.searchsorted(colr2, np.arange(d.NSH))
        dpos = np.arange(n_c) - nstart[colr2]
        pn = (colr2 & 127).astype(np.int64)
        fi = (colr2 >> 7) * d.D + dpos
        curd = np.zeros((128, d.WD), np.float32)
        curd[pn, fi] = cur2
        maskd = np.zeros((128, d.WD), np.float32)
        maskd[pn, fi] = 1.0

        x_pad = np.zeros((d.NPAD, d.FIN), np.float32)
        x_pad[:d.NSH] = x[c * d.NSH:(c + 1) * d.NSH]

        # decoder pair groups (variable size, 4-chunk padded)
        gi1 = gidx(np.asarray(idx[0][c * d.PPC:(c + 1) * d.PPC], np.int64))
        gi2 = gidx(np.asarray(idx[1][c * d.PPC:(c + 1) * d.PPC], np.int64))
        grp = (gi1 // d.SEGR) * NSEG + (gi2 // d.SEGR)
        pkey = np.argsort(grp, kind="stable")
        gcnt = np.bincount(grp, minlength=16)
        gstart = np.zeros(16, np.int64)
        gstart[1:] = np.cumsum(gcnt)[:-1]
        ppos = np.arange(d.PPC) - gstart[grp[pkey]]
        slot = d.gbase[grp[pkey]] * 128 + ppos      # slot in padded layout
        o1a = np.full((128, d.PCH2), PAD_IDX, np.int64)
        o2a = np.full((128, d.PCH2), PAD_IDX, np.int64)
        for g in range(16):
            sl = np.arange(gcnt[g], d.PGM[g]) + d.gbase[g] * 128
            o1a[sl & 127, sl >> 7] = 0
            o2a[sl & 127, sl >> 7] = 0
        o1a[slot & 127, slot >> 7] = gi1[pkey] % d.SEGR
        o2a[slot & 127, slot >> 7] = gi2[pkey] % d.SEGR
        o1g = _wrap16(o1a.astype(np.int16))
        o2g = _wrap16(o2a.astype(np.int16))
        pk = np.empty(d.PPC, np.int64)
        pk[:] = pkey
        perms.append((pk, np.asarray(slot)))

        maps.append(dict(
            x_pad=x_pad, curb=curb, dstf=dstf, g16=g16,
            curd=curd, maskd=maskd, o1g=o1g, o2g=o2g,
            iota_f=iota_f, b_mat=b_mat, linw=linw_rep, linb=linb_rep,
        ))
    return maps, perms


def build(d):
    nc = bacc.Bacc("TRN2", target_bir_lowering=False, debug=False,
                   num_devices=NCORES)
    H, D, W = d.H, d.D, d.W

    x_pad = nc.dram_tensor("x_pad", [d.NPAD, d.FIN], F32, kind="ExternalInput")
    curb = nc.dram_tensor("curb", [128, d.NCH], F32, kind="ExternalInput")
    dstf = nc.dram_tensor("dstf", [128, d.NCH], F32, kind="ExternalInput")
    g16 = nc.dram_tensor("g16", [128, d.NCH * 8], I16, kind="ExternalInput")
    curd = nc.dram_tensor("curd", [128, d.WD], F32, kind="ExternalInput")
    maskd = nc.dram_tensor("maskd", [128, d.WD], F32, kind="ExternalInput")
    o1g = nc.dram_tensor("o1g", [128, d.PCH2 * 8], I16, kind="ExternalInput")
    o2g = nc.dram_tensor("o2g", [128, d.PCH2 * 8], I16, kind="ExternalInput")

    iota_f = nc.dram_tensor("iota_f", [128, 128], F32, kind="ExternalInput")
    b_mat = nc.dram_tensor("b_mat", [128, H], F32, kind="ExternalInput")
    linw = nc.dram_tensor("linw", [128, 10], F32, kind="ExternalInput")
    linb = nc.dram_tensor("linb", [128, 1], F32, kind="ExternalInput")
    w_gcn = nc.dram_tensor("w_gcn", [d.FIN, H], F32, kind="ExternalInput")
    dec1_w = nc.dram_tensor("dec1_w", [4 * H, d.D1], F32, kind="ExternalInput")
    dec1_b = nc.dram_tensor("dec1_b", [d.D1], F32, kind="ExternalInput")
    dec2_w = nc.dram_tensor("dec2_w", [d.D1, 1], F32, kind="ExternalInput")
    dec2_b = nc.dram_tensor("dec2_b", [1], F32, kind="ExternalInput")
    out_d = nc.dram_tensor("out_d", [d.PCH2 * 128, 1], F32, kind="ExternalOutput")

    AT = mybir.ActivationFunctionType
    OP = mybir.AluOpType
    MB = d.D1 // 128

    with ExitStack() as ctx:
        tc = ctx.enter_context(tile.TileContext(nc))
        const = ctx.enter_context(tc.tile_pool(name="const", bufs=1))
        sb = ctx.enter_context(tc.tile_pool(name="sb", bufs=3))
        big = ctx.enter_context(tc.tile_pool(name="big", bufs=3))
        gp = ctx.enter_context(tc.tile_pool(name="gp", bufs=5))   # gather tiles
        ps = ctx.enter_context(tc.tile_pool(name="ps", bufs=2, space="PSUM"))
        dr = ctx.enter_context(tc.tile_pool(name="dr", bufs=1, space="DRAM"))

        # ---------- constants ----------
        iota_sb = const.tile([128, 128], F32, tag="iota32")
        nc.sync.dma_start(out=iota_sb[:], in_=iota_f.ap())
        iota_bf = const.tile([128, 128], BF16, tag="iotabf")
        nc.vector.tensor_copy(out=iota_bf[:], in_=iota_sb[:])
        ident = const.tile([128, 128], BF16, tag="ident")
        make_identity(nc, ident[:])
        ident_f = const.tile([128, 128], F32, tag="identf")
        make_identity(nc, ident_f[:])
        bmat_sb = const.tile([128, H], F32, tag="bmat")
        nc.sync.dma_start(out=bmat_sb[:], in_=b_mat.ap())

        linw_sb = const.tile([128, 10], F32, tag="linw")
        nc.sync.dma_start(out=linw_sb[:], in_=linw.ap())
        linb_sb = const.tile([128, 1], F32, tag="linb")
        nc.sync.dma_start(out=linb_sb[:], in_=linb.ap())
        coef = const.tile([128, 10], F32, tag="coef")
        nc.vector.tensor_scalar_mul(coef[:], linw_sb[:], 0.5)
        csum = const.tile([128, 1], F32, tag="csum")
        nc.vector.tensor_reduce(out=csum[:], in_=coef[:], axis=mybir.AxisListType.X,
                                op=OP.add)
        cconst = const.tile([128, 1], F32, tag="cconst")
        nc.vector.tensor_add(cconst[:], csum[:], linb_sb[:])

        wg_sb = const.tile([128, d.KF, H], F32, tag="wg32")
        nc.sync.dma_start(out=wg_sb[:], in_=w_gcn.ap().rearrange("(k p) h -> p k h", p=128))
        wgr = const.tile([128, d.KF, H], F32R, tag="wgr")
        nc.vector.tensor_copy(out=wgr[:], in_=wg_sb[:])

        d1_sb = big.tile([128, 4, d.D1], F32, tag="big")
        nc.sync.dma_start(out=d1_sb[:], in_=dec1_w.ap().rearrange("(b p) d -> p b d", p=128))
        wc_sb = const.tile([128, 3, d.D1], BF16, tag="wcf")
        nc.vector.tensor_add(wc_sb[:, 0, :], d1_sb[:, 0, :], d1_sb[:, 2, :])
        nc.vector.tensor_add(wc_sb[:, 1, :], d1_sb[:, 0, :], d1_sb[:, 3, :])
        nc.vector.tensor_copy(out=wc_sb[:, 2, :], in_=d1_sb[:, 1, :])

        d1b_sb = const.tile([128, MB], F32, tag="d1b")
        nc.sync.dma_start(out=d1b_sb[:], in_=dec1_b.ap().rearrange("(b p) -> p b", p=128))
        d2_sb = const.tile([128, MB, 1], F32, tag="d232")
        nc.sync.dma_start(out=d2_sb[:], in_=dec2_w.ap().rearrange("(b p) o -> p b o", p=128))
        d2r = const.tile([128, MB, 1], BF16, tag="d2r")
        nc.vector.tensor_copy(out=d2r[:], in_=d2_sb[:])
        d2b_sb = const.tile([1, 1], F32, tag="d2b")
        nc.sync.dma_start(out=d2b_sb[:], in_=dec2_b.ap()[:, None])

        zero_c = const.tile([128, d.CSMAX, H], BF16, tag="zeroc")
        nc.vector.memset(zero_c[:], 0.0)

        # ---------- edge weights (chunk layout) ----------
        def horner(src_ap, n):
            t = big.tile([128, n], F32, tag="big")
            nc.scalar.activation(out=t[:], in_=src_ap, func=AT.Exp, scale=-1.0)
            acc = big.tile([128, n], F32, tag="big")
            nc.vector.tensor_scalar_mul(acc[:], t[:], coef[:, 9:10])
            for k in range(8, -1, -1):
                nc.vector.scalar_tensor_tensor(
                    out=acc[:], in0=acc[:], scalar=coef[:, k:k + 1], in1=t[:],
                    op0=OP.add, op1=OP.mult)
            nc.vector.tensor_scalar_add(acc[:], acc[:], cconst[:])
            return acc

        curb_sb = big.tile([128, d.NCH], F32, tag="big")
        nc.sync.dma_start(out=curb_sb[:], in_=curb.ap())
        ew_nch = horner(curb_sb[:], d.NCH)
        ew_bf = const.tile([128, d.NCH], BF16, tag="ewf")
        nc.vector.tensor_copy(out=ew_bf[:], in_=ew_nch[:])

        dst_sb = big.tile([128, d.NCH], F32, tag="big")
        nc.sync.dma_start(out=dst_sb[:], in_=dstf.ap())
        dst_bf = const.tile([128, d.NCH], BF16, tag="dstbf")
        nc.vector.tensor_copy(out=dst_bf[:], in_=dst_sb[:])

        # ---------- degrees ----------
        WHF = d.WD // d.WH
        WHW = W // d.WH
        deg_r = const.tile([128, W], F32, tag="deg")
        for h in range(d.WH):
            cu = big.tile([128, WHF], F32, tag="big")
            nc.sync.dma_start(out=cu[:], in_=curd.ap()[:, h * WHF:(h + 1) * WHF])
            ewd = horner(cu[:], WHF)
            mk = big.tile([128, WHF], F32, tag="big")
            nc.sync.dma_start(out=mk[:], in_=maskd.ap()[:, h * WHF:(h + 1) * WHF])
            nc.vector.tensor_mul(ewd[:], ewd[:], mk[:])
            nc.vector.tensor_reduce(
                out=deg_r[:, h * WHW:(h + 1) * WHW],
                in_=ewd[:].rearrange("p (w dd) -> p w dd", dd=D),
                axis=mybir.AxisListType.X, op=OP.add)
        nc.vector.tensor_scalar_add(deg_r[:], deg_r[:], 1.0)
        mw = const.tile([128, W], F32, tag="mw")
        nc.vector.tensor_single_scalar(out=mw[:], in_=deg_r[:], scalar=0.0, op=OP.is_gt)
        degm = const.tile([128, W], F32, tag="degm")
        nc.vector.tensor_mul(degm[:], deg_r[:], mw[:])
        onem = const.tile([128, W], F32, tag="onem")
        nc.vector.tensor_scalar(out=onem[:], in0=mw[:], scalar1=-1.0, scalar2=1.0,
                                op0=OP.mult, op1=OP.add)
        nc.vector.tensor_add(degm[:], degm[:], onem[:])
        rec = const.tile([128, W], F32, tag="rec")
        nc.vector.reciprocal(out=rec[:], in_=degm[:])
        dsq = const.tile([128, W], F32, tag="dsq")
        nc.scalar.activation(out=dsq[:], in_=rec[:], func=AT.Sqrt)
        dinv = const.tile([128, W], F32, tag="dinv")
        nc.vector.tensor_mul(dinv[:], dsq[:], mw[:])

        # ---------- xw + y per window ----------
        y_loc = dr.tile([d.NPAD, H], BF16)
        y_loc_r = y_loc[:].rearrange("(w p) h -> p w h", p=128)
        for w in range(W):
            xc = sb.tile([128, d.FIN], F32, tag="xc")
            nc.sync.dma_start(out=xc[:], in_=x_pad.ap()[w * 128:(w + 1) * 128, :])
            xt = sb.tile([128, d.KF, 128], F32R, tag="xt")
            for k in range(d.KF):
                pt = ps.tile([128, 128], F32, tag="trf")
                nc.tensor.transpose(pt[:], xc[:, k * 128:(k + 1) * 128], ident_f[:])
                nc.scalar.copy(out=xt[:, k, :], in_=pt[:])
            pxw = ps.tile([128, H], F32, tag="win")
            for k in range(d.KF):
                nc.tensor.matmul(pxw[:], lhsT=xt[:, k, :], rhs=wgr[:, k, :],
                                 start=(k == 0), stop=(k == d.KF - 1))
            yw = sb.tile([128, H], BF16, tag="yw")
            nc.scalar.activation(out=yw[:], in_=pxw[:], func=AT.Copy,
                                 scale=dinv[:, w:w + 1])
            nc.sync.dma_start(out=y_loc_r[:, w, :], in_=yw[:])

        y_full = dr.tile([NCORES * d.NPAD, H], BF16, addr_space="Shared")
        nc.gpsimd.collective_compute(
            "AllGather", OP.bypass, replica_groups=[list(range(NCORES))],
            ins=[y_loc[:]], outs=[y_full[:]])

        # ---------- aggregation ----------
        x1_loc = dr.tile([d.NPAD, H], BF16)
        x1_loc_r = x1_loc[:].rearrange("(w p) h -> p w h", p=128)
        for w in range(W):
            cb = d.colbase[w]
            # one idx load covering this window's 4 segment calls
            ix = sb.tile([128, d.WS[w] * 8], I16, tag="ix")
            nc.sync.dma_start(out=ix[:], in_=g16.ap()[:, cb[0] * 8:(cb[0] + d.WS[w]) * 8])
            yts = []
            for s in range(NSEG):
                cs = d.CS_tab[w][s]
                if cs == 0:
                    yts.append(None)
                    continue
                off = cb[s] - cb[0]
                yt = gp.tile([128, cs, H], BF16, tag="yt")
                nc.scalar.copy(out=yt[:], in_=zero_c[:, :cs, :])
                nc.gpsimd.dma_gather(
                    out_ap=yt[:], in_ap=y_full[s * d.SEGR:(s + 1) * d.SEGR, :],
                    idxs_ap=ix[:, off * 8:(off + cs) * 8],
                    num_idxs=cs * 128, num_idxs_reg=d.M_tab[w][s],
                    elem_size=H, single_packet=False)
                yts.append(yt)
            pw = ps.tile([128, H], F32, tag="win")
            first = True
            for s in range(NSEG):
                cs = d.CS_tab[w][s]
                if cs == 0:
                    continue
                c0 = cb[s]
                # S'[p,k,j] = (iota_j == dst[p,k]) * ew[p,k]; then
                # pw += S'^T @ y_rows fuses the edge-weight multiply into PE.
                ss = sb.tile([128, cs, 128], BF16, tag="sc")
                for k in range(cs):
                    nc.vector.scalar_tensor_tensor(
                        out=ss[:, k, :], in0=iota_bf[:],
                        scalar=dst_bf[:, c0 + k:c0 + k + 1],
                        in1=ew_bf[:, c0 + k, None].to_broadcast([128, 128]),
                        op0=OP.is_equal, op1=OP.mult)
                for k in range(cs):
                    last = (s == NSEG - 1 or sum(d.CS_tab[w][s + 1:]) == 0) \
                        and (k == cs - 1)
                    nc.tensor.matmul(pw[:], lhsT=ss[:, k, :],
                                     rhs=yts[s][:, k, :],
                                     start=first, stop=last)
                    first = False
            ywr = sb.tile([128, H], BF16, tag="ywr")
            nc.sync.dma_start(out=ywr[:], in_=y_loc_r[:, w, :])
            t1 = sb.tile([128, H], F32, tag="t1")
            nc.vector.tensor_add(t1[:], pw[:], ywr[:])
            t2 = sb.tile([128, H], F32, tag="t2")
            nc.vector.scalar_tensor_tensor(
                out=t2[:], in0=t1[:], scalar=dinv[:, w:w + 1], in1=bmat_sb[:],
                op0=OP.mult, op1=OP.add)
            x1w = sb.tile([128, H], BF16, tag="x1w")
            nc.scalar.activation(out=x1w[:], in_=t2[:], func=AT.Relu)
            nc.sync.dma_start(out=x1_loc_r[:, w, :], in_=x1w[:])

        x1_full = dr.tile([NCORES * d.NPAD, H], BF16, addr_space="Shared")
        nc.gpsimd.collective_compute(
            "AllGather", OP.bypass, replica_groups=[list(range(NCORES))],
            ins=[x1_loc[:]], outs=[x1_full[:]])

        # ---------- pair decoder ----------
        out_r = out_d.ap().rearrange("a b -> b a")      # [1, PCH2*128]
        for g in range(16):
            PGg = d.PG_tab[g]
            gb = int(d.gbase[g])
            a_seg, b_seg = g // NSEG, g % NSEG
            e1 = gp.tile([128, PGg, H], BF16, tag="et")
            e2 = gp.tile([128, PGg, H], BF16, tag="et")
            for (tile_, src_seg, arr) in ((e1, a_seg, o1g), (e2, b_seg, o2g)):
                ixd = sb.tile([128, PGg * 8], I16, tag="ixd")
                nc.sync.dma_start(out=ixd[:], in_=arr.ap()[:, gb * 8:(gb + PGg) * 8])
                nc.gpsimd.dma_gather(
                    out_ap=tile_[:],
                    in_ap=x1_full[src_seg * d.SEGR:(src_seg + 1) * d.SEGR, :],
                    idxs_ap=ixd[:], num_idxs=PGg * 128,
                    num_idxs_reg=d.PGM[g],
                    elem_size=H, single_packet=False)
            em = gp.tile([128, PGg, H], BF16, tag="et")
            nc.vector.tensor_mul(em[:], e1[:], e2[:])
            NTg = (PGg + 3) // 4
            ob = sb.tile([1, NTg * 512], F32, tag="ob")
            for nt in range(NTg):
                cT = sb.tile([128, 3, 4, 128], BF16, tag="cT")
                for jj in range(min(4, PGg - nt * 4)):
                    j = nt * 4 + jj
                    for cix, src in enumerate((e1, e2, em)):
                        pt = ps.tile([128, 128], BF16, tag="trf")
                        nc.tensor.transpose(pt[:], src[:, j, :], ident[:])
                        if (j + cix) % 2 == 0:
                            nc.scalar.copy(out=cT[:, cix, jj, :], in_=pt[:])
                        else:
                            nc.vector.tensor_copy(out=cT[:, cix, jj, :], in_=pt[:])
                h_sb = sb.tile([128, MB, 512], BF16, tag="hsb")
                for mb in range(MB):
                    ph = ps.tile([128, 512], F32, tag="ph")
                    for cix in range(3):
                        nc.tensor.matmul(
                            ph[:], lhsT=wc_sb[:, cix, mb * 128:(mb + 1) * 128],
                            rhs=cT[:, cix, :, :],
                            start=(cix == 0), stop=(cix == 2))
                    nc.scalar.activation(out=h_sb[:, mb, :], in_=ph[:], func=AT.Relu,
                                         bias=d1b_sb[:, mb:mb + 1])
                po = ps.tile([1, 512], F32, tag="po")
                for mb in range(MB):
                    nc.tensor.matmul(po[:], lhsT=d2r[:, mb, :],
                                     rhs=h_sb[:, mb, :],
                                     start=(mb == 0), stop=(mb == MB - 1))
                nc.scalar.activation(out=ob[:, nt * 512:(nt + 1) * 512], in_=po[:],
                                     func=AT.Identity, bias=d2b_sb[:, :])
            nc.sync.dma_start(out=out_r[:, gb * 128:(gb + PGg) * 128],
                              in_=ob[:, :PGg * 128])

    nc.compile()
    return nc


_CACHE = {}
TRACE = False          # test harness sets True to capture NTFF profile
LAST_RESULT = None     # BassKernelResults of the most recent run


def kernel(**inputs):
    x = np.asarray(inputs["x"], np.float32)
    curvature = np.asarray(inputs["curvature"], np.float32)
    edge_index = np.asarray(inputs["edge_index"])
    idx = np.asarray(inputs["idx"])
    N, FIN = x.shape
    E = edge_index.shape[1]
    P = idx.shape[1]
    H = np.asarray(inputs["W_gcn"]).shape[1]
    D1 = np.asarray(inputs["dec1_w"]).shape[1]

    CS_tab, D, PG_tab, M_tab, PGM = chunk_params(edge_index, idx, N)
    d = Dims(N, E, P, FIN, H, D1, CS_tab, D, PG_tab, M_tab, PGM)
    maps, perms = preprocess(d, x, curvature, edge_index, idx,
                             inputs["b_gcn"], inputs["lin1_w"], inputs["lin1_b"])
    shared = dict(
        w_gcn=np.asarray(inputs["W_gcn"], np.float32),
        dec1_w=np.asarray(inputs["dec1_w"], np.float32),
        dec1_b=np.asarray(inputs["dec1_b"], np.float32).reshape(-1),
        dec2_w=np.asarray(inputs["dec2_w"], np.float32),
        dec2_b=np.asarray(inputs["dec2_b"], np.float32).reshape(-1),
    )
    for m in maps:
        m.update(shared)

    key = (N, E, P, tuple(tuple(r) for r in CS_tab), D, tuple(PG_tab),
           tuple(tuple(r) for r in M_tab), tuple(PGM))
    if key not in _CACHE:
        _CACHE[key] = build(d)
    nc = _CACHE[key]

    from concourse.bass_utils import run_bass_kernel_spmd
    res = run_bass_kernel_spmd(nc, maps, core_ids=list(range(NCORES)),
                               trace=TRACE)
    global LAST_RESULT
    LAST_RESULT = res
    out = np.empty((P, 1), np.float32)
    for c in range(NCORES):
        vals = np.asarray(res.results[c]["out_d"], np.float32)[:, 0]
        pk, slot = perms[c]
        out[c * d.PPC + pk, 0] = vals[slot]
    return out


# revision 41
# speedup vs baseline: 1.9725x; 1.9725x over previous
"""Trainium2 Bass kernel for nn_CGCN (curvature-weighted GCN + pair decoder).

Strategy (8 NeuronCores, SPMD):
  - Edges sharded by DESTINATION node: core c owns nodes [c*N/8, (c+1)*N/8)
    and every edge whose col lands there (host bins/sorts; index plumbing
    only). The scatter-add stays core-local - no [N,H] all-reduce.
  - Edge weights ew = Linear(func_k(curvature)) on device via Horner in
    t = exp(-c).
  - Degrees: per-node padded layout [node-on-partition, window x Dmax],
    fp32 vector reduce (exact per-node sums, no long-prefix cancellation).
  - y = dinv * (x @ W) per 128-node window (bf16), AllGathered.
  - Aggregation: per 128-edge chunk, S[p,j] = (dst_rel[p]==j) built on DVE;
    psum[window,H] += S^T @ (y_rows * ew) on PE. y rows fetched with
    dma_gather (int16 idx), the table split in 4 segments of 2*NPAD rows so
    indices fit int16. Chunk counts are per-(window, segment) (max over the
    8 cores - SPMD shares one program); per-core shortfall is padded with
    idx=-1, which the gather ucode trims from descriptor generation, so
    padding costs no Q7 time. Gather tiles are pre-zeroed on the Scalar
    engine so trimmed (stale) slots can never inject NaN into the matmul.
  - x1 = relu(dinv*(agg+y)+b) in bf16, AllGathered (halves collective and
    decoder-gather bytes vs f32).
  - Decoder: pairs grouped by (seg(e1), seg(e2)) into 16 variable-size
    groups (chunks padded to a multiple of 4); dma_gather idx padded with
    -1 (trimmed; NaN from stale slots stays column-isolated and those
    columns are discarded by the host un-permute). feat@dec1_w decomposed
    as e1@(Wa+Wc) + e2@(Wa+Wd) + (e1*e2)@Wb, all in bf16 (F32R matmuls are
    ~4x slower on PE).
"""
import sys

for _p in ("/opt/trn_rl_repo",):
    if _p not in sys.path:
        sys.path.append(_p)

import numpy as np
from contextlib import ExitStack

import concourse.bass as bass
import concourse.tile as tile
from concourse import mybir, bacc
from concourse.masks import make_identity

F32 = mybir.dt.float32
F32R = mybir.dt.float32r
BF16 = mybir.dt.bfloat16
I32 = mybir.dt.int32
I16 = mybir.dt.int16

NCORES = 8
NSEG = 4          # table segments (2 cores' rows each) so idx fits int16
WARM_W = 0        # gather tiles are pre-zeroed instead of warmed
PAD_IDX = -1      # -1: ucode trims trailing pads (no Q7 cost); 0: plain pad


def _wrap16(a):
    """[128, n] per-slot idx array -> dma_gather idx layout [128, n*8] int16.

    Slot (chunk c, partition p) maps to idx-list position i = c*128 + p;
    the ucode reads position i from [p16=i%16, col=c*8 + (i%128)//16],
    partitions replicated x8.
    """
    p128, n = a.shape
    assert p128 == 128
    m = np.zeros((16, n, 8), np.int16)
    for s16 in range(8):
        m[:, :, s16] = a[s16 * 16:(s16 + 1) * 16, :]
    m = m.reshape(16, n * 8)
    return np.tile(m, (8, 1)).copy()


class Dims:
    def __init__(self, N, E, P, FIN, H, D1, CS_tab, D, PG_tab, M_tab, PGM):
        self.N, self.E, self.P = N, E, P
        self.FIN, self.H, self.D1 = FIN, H, D1
        self.CS_tab = CS_tab           # [W][NSEG] chunks per (window, seg)
        self.D = D                     # max node in-degree
        self.PG_tab = PG_tab           # [16] decoder chunks per pair-group
        self.M_tab = M_tab             # [W][NSEG] max-over-cores idx count
        self.PGM = PGM                 # [16] max-over-cores pair count
        self.NSH = N // NCORES
        self.W = (self.NSH + 127) // 128
        self.NPAD = self.W * 128
        self.SEGR = 2 * self.NPAD      # rows per table segment
        assert len(CS_tab) == self.W
        # chunk-column base per (w, s); layout is w-major then s
        self.colbase = np.zeros((self.W, NSEG), np.int64)
        acc = 0
        for w in range(self.W):
            for s in range(NSEG):
                self.colbase[w, s] = acc
                acc += CS_tab[w][s]
        self.NCH = acc                 # total chunk columns
        self.WS = [sum(CS_tab[w]) for w in range(self.W)]
        self.CSMAX = max(max(CS_tab[w]) for w in range(self.W))
        self.WD = self.W * D
        self.PPC = P // NCORES
        # decoder groups (chunks, multiple of 4)
        self.gbase = np.zeros(16, np.int64)
        acc = 0
        for g in range(16):
            self.gbase[g] = acc
            acc += PG_tab[g]
        self.PCH2 = acc
        self.KF = FIN // 128
        self.WH = 2 if self.W % 2 == 0 else 1
        assert self.SEGR <= 32768


def chunk_params(edge_index, idx, N):
    """Per-(window,seg) chunk counts / D / decoder group sizes (max over cores)."""
    NSH = N // NCORES
    W = (NSH + 127) // 128
    NPAD = W * 128
    SEGR = 2 * NPAD
    col = np.asarray(edge_index[1], np.int64)
    row = np.asarray(edge_index[0], np.int64)
    order = np.argsort(col, kind="stable")
    col_s, row_s = col[order], row[order]
    D = int(np.bincount(col_s, minlength=N).max())
    seg_of_row = ((row_s // NSH) * NPAD + (row_s % NSH)) // SEGR
    cs_tab = np.zeros((W, NSEG), np.int64)
    m_tab = np.zeros((W, NSEG), np.int64)
    for c in range(NCORES):
        lo, hi = np.searchsorted(col_s, [c * NSH, (c + 1) * NSH])
        wof = (col_s[lo:hi] - c * NSH) >> 7
        sof = seg_of_row[lo:hi]
        cnt = np.bincount(wof * NSEG + sof, minlength=W * NSEG).reshape(W, NSEG)
        cs_tab = np.maximum(cs_tab, (cnt + 127) // 128)
        m_tab = np.maximum(m_tab, cnt)
    cs_tab[:, 0] = np.maximum(cs_tab[:, 0], 1)   # >=1 matmul per window
    CS_tab = [[int(x) for x in cs_tab[w]] for w in range(W)]
    M_tab = [[int(x) for x in m_tab[w]] for w in range(W)]
    # decoder groups
    i1 = np.asarray(idx[0], np.int64)
    i2 = np.asarray(idx[1], np.int64)
    g1 = ((i1 // NSH) * NPAD + (i1 % NSH)) // SEGR
    g2 = ((i2 // NSH) * NPAD + (i2 % NSH)) // SEGR
    PPC = i1.shape[0] // NCORES
    pg = np.zeros(16, np.int64)
    pgm = np.zeros(16, np.int64)
    for c in range(NCORES):
        g = g1[c * PPC:(c + 1) * PPC] * NSEG + g2[c * PPC:(c + 1) * PPC]
        cnt = np.bincount(g, minlength=16)
        pg = np.maximum(pg, (cnt + 127) // 128)
        pgm = np.maximum(pgm, cnt)
    pg = np.maximum(pg, 1)
    PG_tab = [int(x) for x in pg]
    PGM = [int(x) for x in pgm]
    return CS_tab, D, PG_tab, M_tab, PGM


def preprocess(d, x, curvature, edge_index, idx, b_gcn, lin1_w, lin1_b):
    """Index plumbing: sort/bin/pad edges and pairs, build per-core inputs."""
    row = np.asarray(edge_index[0], dtype=np.int64)
    col = np.asarray(edge_index[1], dtype=np.int64)
    cur = np.asarray(curvature, dtype=np.float32)
    order = np.argsort(col, kind="stable")
    row_s, col_s, cur_s = row[order], col[order], cur[order]
    bounds = np.searchsorted(col_s, np.arange(NCORES + 1) * d.NSH)

    iota_f = np.broadcast_to(np.arange(128, dtype=np.float32), (128, 128)).copy()
    b_mat = np.broadcast_to(np.asarray(b_gcn, np.float32)[None, :], (128, d.H)).copy()
    linw_rep = np.broadcast_to(np.asarray(lin1_w, np.float32)[:, 0][None, :], (128, 10)).copy()
    linb_rep = np.full((128, 1), np.float32(np.asarray(lin1_b).reshape(-1)[0]), np.float32)

    def gidx(nodes):
        return ((nodes // d.NSH) * d.NPAD + (nodes % d.NSH)).astype(np.int64)

    maps = []
    perms = []
    for c in range(NCORES):
        lo, hi = bounds[c], bounds[c + 1]
        colr = (col_s[lo:hi] - c * d.NSH).astype(np.int64)
        grow = gidx(row_s[lo:hi])
        curc = cur_s[lo:hi]
        n_c = colr.shape[0]
        wof = colr >> 7
        sof = grow // d.SEGR
        # order edges by (window, segment) [stable within = by col]
        okey = np.argsort(wof * NSEG + sof, kind="stable")
        colr, grow, curc = colr[okey], grow[okey], curc[okey]
        wof, sof = wof[okey], sof[okey]
        # chunk column for each edge
        cnt = np.bincount(wof * NSEG + sof, minlength=d.W * NSEG).reshape(d.W, NSEG)
        start = np.zeros(d.W * NSEG, np.int64)
        start[1:] = np.cumsum(cnt.reshape(-1))[:-1]
        pos = np.arange(n_c) - start[wof * NSEG + sof]
        ci = d.colbase[wof, sof] + (pos >> 7)
        p = (pos & 127).astype(np.int64)

        dstf = np.full((128, d.NCH), -1.0, np.float32)
        dstf[p, ci] = (colr & 127).astype(np.float32)
        curb = np.zeros((128, d.NCH), np.float32)
        curb[p, ci] = curc
        # idx padding: zeros up to the shared per-call count M_tab (so the
        # static num_idxs_reg matches the ucode's trailing-negative trim on
        # every core), then -1 to chunk capacity (trimmed, no Q7 cost).
        ridx = np.full((128, d.NCH), PAD_IDX, np.int64)
        for w in range(d.W):
            for s in range(NSEG):
                cs = d.CS_tab[w][s]
                if cs == 0:
                    continue
                c0 = d.colbase[w, s]
                m = d.M_tab[w][s]
                sl = np.arange(cnt[w, s], m)
                ridx[sl & 127, c0 + (sl >> 7)] = 0
        ridx[p, ci] = grow - sof * d.SEGR
        g16 = _wrap16(ridx.astype(np.int16))

        # degree layout (per-node padded)
        colr2 = col_s[lo:hi] - c * d.NSH
        cur2 = cur_s[lo:hi]
        nstart = np.searchsorted(colr2, np.arange(d.NSH))
        dpos = np.arange(n_c) - nstart[colr2]
        pn = (colr2 & 127).astype(np.int64)
        fi = (colr2 >> 7) * d.D + dpos
        curd = np.zeros((128, d.WD), np.float32)
        curd[pn, fi] = cur2
        maskd = np.zeros((128, d.WD), np.float32)
        maskd[pn, fi] = 1.0

        x_pad = np.zeros((d.NPAD, d.FIN), np.float32)
        x_pad[:d.NSH] = x[c * d.NSH:(c + 1) * d.NSH]
        xt_pad = np.ascontiguousarray(x_pad.T).reshape(d.KF, 128, d.NPAD)

        # decoder pair groups (variable size, 4-chunk padded)
        gi1 = gidx(np.asarray(idx[0][c * d.PPC:(c + 1) * d.PPC], np.int64))
        gi2 = gidx(np.asarray(idx[1][c * d.PPC:(c + 1) * d.PPC], np.int64))
        grp = (gi1 // d.SEGR) * NSEG + (gi2 // d.SEGR)
        pkey = np.argsort(grp, kind="stable")
        gcnt = np.bincount(grp, minlength=16)
        gstart = np.zeros(16, np.int64)
        gstart[1:] = np.cumsum(gcnt)[:-1]
        ppos = np.arange(d.PPC) - gstart[grp[pkey]]
        slot = d.gbase[grp[pkey]] * 128 + ppos      # slot in padded layout
        o1a = np.full((128, d.PCH2), PAD_IDX, np.int64)
        o2a = np.full((128, d.PCH2), PAD_IDX, np.int64)
        for g in range(16):
            sl = np.arange(gcnt[g], d.PGM[g]) + d.gbase[g] * 128
            o1a[sl & 127, sl >> 7] = 0
            o2a[sl & 127, sl >> 7] = 0
        o1a[slot & 127, slot >> 7] = gi1[pkey] % d.SEGR
        o2a[slot & 127, slot >> 7] = gi2[pkey] % d.SEGR
        o1g = _wrap16(o1a.astype(np.int16))
        o2g = _wrap16(o2a.astype(np.int16))
        pk = np.empty(d.PPC, np.int64)
        pk[:] = pkey
        perms.append((pk, np.asarray(slot)))

        maps.append(dict(
            xt_pad=xt_pad, curb=curb, dstf=dstf, g16=g16,
            curd=curd, maskd=maskd, o1g=o1g, o2g=o2g,
            iota_f=iota_f, b_mat=b_mat, linw=linw_rep, linb=linb_rep,
        ))
    return maps, perms


def build(d):
    nc = bacc.Bacc("TRN2", target_bir_lowering=False, debug=False,
                   num_devices=NCORES, num_swdge_queues=4)
    H, D, W = d.H, d.D, d.W
    qrr = [0]          # round-robin SWDGE queue so gathers use all Q7 pairs

    def next_q():
        q = qrr[0] % 4
        qrr[0] += 1
        return q

    xt_pad = nc.dram_tensor("xt_pad", [d.KF, 128, d.NPAD], F32R,
                            kind="ExternalInput")
    curb = nc.dram_tensor("curb", [128, d.NCH], F32, kind="ExternalInput")
    dstf = nc.dram_tensor("dstf", [128, d.NCH], F32, kind="ExternalInput")
    g16 = nc.dram_tensor("g16", [128, d.NCH * 8], I16, kind="ExternalInput")
    curd = nc.dram_tensor("curd", [128, d.WD], F32, kind="ExternalInput")
    maskd = nc.dram_tensor("maskd", [128, d.WD], F32, kind="ExternalInput")
    o1g = nc.dram_tensor("o1g", [128, d.PCH2 * 8], I16, kind="ExternalInput")
    o2g = nc.dram_tensor("o2g", [128, d.PCH2 * 8], I16, kind="ExternalInput")

    iota_f = nc.dram_tensor("iota_f", [128, 128], F32, kind="ExternalInput")
    b_mat = nc.dram_tensor("b_mat", [128, H], F32, kind="ExternalInput")
    linw = nc.dram_tensor("linw", [128, 10], F32, kind="ExternalInput")
    linb = nc.dram_tensor("linb", [128, 1], F32, kind="ExternalInput")
    w_gcn = nc.dram_tensor("w_gcn", [d.FIN, H], F32, kind="ExternalInput")
    dec1_w = nc.dram_tensor("dec1_w", [4 * H, d.D1], F32, kind="ExternalInput")
    dec1_b = nc.dram_tensor("dec1_b", [d.D1], F32, kind="ExternalInput")
    dec2_w = nc.dram_tensor("dec2_w", [d.D1, 1], F32, kind="ExternalInput")
    dec2_b = nc.dram_tensor("dec2_b", [1], F32, kind="ExternalInput")
    out_d = nc.dram_tensor("out_d", [d.PCH2 * 128, 1], F32, kind="ExternalOutput")

    AT = mybir.ActivationFunctionType
    OP = mybir.AluOpType
    MB = d.D1 // 128

    with ExitStack() as ctx:
        tc = ctx.enter_context(tile.TileContext(nc))
        const = ctx.enter_context(tc.tile_pool(name="const", bufs=1))
        sb = ctx.enter_context(tc.tile_pool(name="sb", bufs=3))
        big = ctx.enter_context(tc.tile_pool(name="big", bufs=3))
        gp = ctx.enter_context(tc.tile_pool(name="gp", bufs=5))   # gather tiles
        ps = ctx.enter_context(tc.tile_pool(name="ps", bufs=2, space="PSUM"))
        dr = ctx.enter_context(tc.tile_pool(name="dr", bufs=1, space="DRAM"))

        # ---------- constants ----------
        iota_sb = const.tile([128, 128], F32, tag="iota32")
        nc.sync.dma_start(out=iota_sb[:], in_=iota_f.ap())
        iota_bf = const.tile([128, 128], BF16, tag="iotabf")
        nc.vector.tensor_copy(out=iota_bf[:], in_=iota_sb[:])
        ident = const.tile([128, 128], BF16, tag="ident")
        make_identity(nc, ident[:])
        ident_f = const.tile([128, 128], F32, tag="identf")
        make_identity(nc, ident_f[:])
        bmat_sb = const.tile([128, H], F32, tag="bmat")
        nc.sync.dma_start(out=bmat_sb[:], in_=b_mat.ap())

        linw_sb = const.tile([128, 10], F32, tag="linw")
        nc.sync.dma_start(out=linw_sb[:], in_=linw.ap())
        linb_sb = const.tile([128, 1], F32, tag="linb")
        nc.sync.dma_start(out=linb_sb[:], in_=linb.ap())
        coef = const.tile([128, 10], F32, tag="coef")
        nc.vector.tensor_scalar_mul(coef[:], linw_sb[:], 0.5)
        csum = const.tile([128, 1], F32, tag="csum")
        nc.vector.tensor_reduce(out=csum[:], in_=coef[:], axis=mybir.AxisListType.X,
                                op=OP.add)
        cconst = const.tile([128, 1], F32, tag="cconst")
        nc.vector.tensor_add(cconst[:], csum[:], linb_sb[:])

        wg_sb = const.tile([128, d.KF, H], F32, tag="wg32")
        nc.sync.dma_start(out=wg_sb[:], in_=w_gcn.ap().rearrange("(k p) h -> p k h", p=128))
        wgr = const.tile([128, d.KF, H], F32R, tag="wgr")
        nc.vector.tensor_copy(out=wgr[:], in_=wg_sb[:])

        d1_sb = big.tile([128, 4, d.D1], F32, tag="big")
        nc.sync.dma_start(out=d1_sb[:], in_=dec1_w.ap().rearrange("(b p) d -> p b d", p=128))
        wc_sb = const.tile([128, 3, d.D1], BF16, tag="wcf")
        nc.vector.tensor_add(wc_sb[:, 0, :], d1_sb[:, 0, :], d1_sb[:, 2, :])
        nc.vector.tensor_add(wc_sb[:, 1, :], d1_sb[:, 0, :], d1_sb[:, 3, :])
        nc.vector.tensor_copy(out=wc_sb[:, 2, :], in_=d1_sb[:, 1, :])

        d1b_sb = const.tile([128, MB], F32, tag="d1b")
        nc.sync.dma_start(out=d1b_sb[:], in_=dec1_b.ap().rearrange("(b p) -> p b", p=128))
        d2_sb = const.tile([128, MB, 1], F32, tag="d232")
        nc.sync.dma_start(out=d2_sb[:], in_=dec2_w.ap().rearrange("(b p) o -> p b o", p=128))
        d2r = const.tile([128, MB, 1], BF16, tag="d2r")
        nc.vector.tensor_copy(out=d2r[:], in_=d2_sb[:])
        d2b_sb = const.tile([1, 1], F32, tag="d2b")
        nc.sync.dma_start(out=d2b_sb[:], in_=dec2_b.ap()[:, None])

        zero_c = const.tile([128, d.CSMAX, H], BF16, tag="zeroc")
        nc.vector.memset(zero_c[:], 0.0)

        # ---------- edge weights (chunk layout) ----------
        def horner(src_ap, n):
            t = big.tile([128, n], F32, tag="big")
            nc.scalar.activation(out=t[:], in_=src_ap, func=AT.Exp, scale=-1.0)
            acc = big.tile([128, n], F32, tag="big")
            nc.vector.tensor_scalar_mul(acc[:], t[:], coef[:, 9:10])
            for k in range(8, -1, -1):
                nc.vector.scalar_tensor_tensor(
                    out=acc[:], in0=acc[:], scalar=coef[:, k:k + 1], in1=t[:],
                    op0=OP.add, op1=OP.mult)
            nc.vector.tensor_scalar_add(acc[:], acc[:], cconst[:])
            return acc

        curb_sb = big.tile([128, d.NCH], F32, tag="big")
        nc.sync.dma_start(out=curb_sb[:], in_=curb.ap())
        ew_nch = horner(curb_sb[:], d.NCH)
        ew_bf = const.tile([128, d.NCH], BF16, tag="ewf")
        nc.vector.tensor_copy(out=ew_bf[:], in_=ew_nch[:])

        dst_sb = big.tile([128, d.NCH], F32, tag="big")
        nc.sync.dma_start(out=dst_sb[:], in_=dstf.ap())
        dst_bf = const.tile([128, d.NCH], BF16, tag="dstbf")
        nc.vector.tensor_copy(out=dst_bf[:], in_=dst_sb[:])

        # ---------- degrees ----------
        WHF = d.WD // d.WH
        WHW = W // d.WH
        deg_r = const.tile([128, W], F32, tag="deg")
        for h in range(d.WH):
            cu = big.tile([128, WHF], F32, tag="big")
            nc.sync.dma_start(out=cu[:], in_=curd.ap()[:, h * WHF:(h + 1) * WHF])
            ewd = horner(cu[:], WHF)
            mk = big.tile([128, WHF], F32, tag="big")
            nc.sync.dma_start(out=mk[:], in_=maskd.ap()[:, h * WHF:(h + 1) * WHF])
            nc.vector.tensor_mul(ewd[:], ewd[:], mk[:])
            nc.vector.tensor_reduce(
                out=deg_r[:, h * WHW:(h + 1) * WHW],
                in_=ewd[:].rearrange("p (w dd) -> p w dd", dd=D),
                axis=mybir.AxisListType.X, op=OP.add)
        nc.vector.tensor_scalar_add(deg_r[:], deg_r[:], 1.0)
        mw = const.tile([128, W], F32, tag="mw")
        nc.vector.tensor_single_scalar(out=mw[:], in_=deg_r[:], scalar=0.0, op=OP.is_gt)
        degm = const.tile([128, W], F32, tag="degm")
        nc.vector.tensor_mul(degm[:], deg_r[:], mw[:])
        onem = const.tile([128, W], F32, tag="onem")
        nc.vector.tensor_scalar(out=onem[:], in0=mw[:], scalar1=-1.0, scalar2=1.0,
                                op0=OP.mult, op1=OP.add)
        nc.vector.tensor_add(degm[:], degm[:], onem[:])
        rec = const.tile([128, W], F32, tag="rec")
        nc.vector.reciprocal(out=rec[:], in_=degm[:])
        dsq = const.tile([128, W], F32, tag="dsq")
        nc.scalar.activation(out=dsq[:], in_=rec[:], func=AT.Sqrt)
        dinv = const.tile([128, W], F32, tag="dinv")
        nc.vector.tensor_mul(dinv[:], dsq[:], mw[:])

        # ---------- xw + y per window ----------
        y_loc = dr.tile([d.NPAD, H], BF16)
        y_loc_r = y_loc[:].rearrange("(w p) h -> p w h", p=128)
        for w in range(W):
            xt = sb.tile([128, d.KF, 128], F32R, tag="xt")
            nc.sync.dma_start(
                out=xt[:],
                in_=xt_pad.ap().rearrange("k p n -> p k n")[:, :, w * 128:(w + 1) * 128])
            pxw = ps.tile([128, H], F32, tag="win")
            for k in range(d.KF):
                nc.tensor.matmul(pxw[:], lhsT=xt[:, k, :], rhs=wgr[:, k, :],
                                 start=(k == 0), stop=(k == d.KF - 1))
            yw = sb.tile([128, H], BF16, tag="yw")
            nc.scalar.activation(out=yw[:], in_=pxw[:], func=AT.Copy,
                                 scale=dinv[:, w:w + 1])
            nc.sync.dma_start(out=y_loc_r[:, w, :], in_=yw[:])

        y_full = dr.tile([NCORES * d.NPAD, H], BF16, addr_space="Shared")
        nc.gpsimd.collective_compute(
            "AllGather", OP.bypass, replica_groups=[list(range(NCORES))],
            ins=[y_loc[:]], outs=[y_full[:]])

        # ---------- aggregation ----------
        x1_loc = dr.tile([d.NPAD, H], BF16)
        x1_loc_r = x1_loc[:].rearrange("(w p) h -> p w h", p=128)
        for w in range(W):
            cb = d.colbase[w]
            # one idx load covering this window's 4 segment calls
            ix = sb.tile([128, d.WS[w] * 8], I16, tag="ix")
            nc.sync.dma_start(out=ix[:], in_=g16.ap()[:, cb[0] * 8:(cb[0] + d.WS[w]) * 8])
            yts = []
            for s in range(NSEG):
                cs = d.CS_tab[w][s]
                if cs == 0:
                    yts.append(None)
                    continue
                off = cb[s] - cb[0]
                yt = gp.tile([128, cs, H], BF16, tag="yt")
                nc.scalar.copy(out=yt[:], in_=zero_c[:, :cs, :])
                nc.gpsimd.dma_gather(
                    out_ap=yt[:], in_ap=y_full[s * d.SEGR:(s + 1) * d.SEGR, :],
                    idxs_ap=ix[:, off * 8:(off + cs) * 8],
                    num_idxs=cs * 128, num_idxs_reg=d.M_tab[w][s],
                    elem_size=H, single_packet=False, queue_num=next_q())
                yts.append(yt)
            pw = ps.tile([128, H], F32, tag="win")
            first = True
            for s in range(NSEG):
                cs = d.CS_tab[w][s]
                if cs == 0:
                    continue
                c0 = cb[s]
                # S'[p,k,j] = (iota_j == dst[p,k]) * ew[p,k]; then
                # pw += S'^T @ y_rows fuses the edge-weight multiply into PE.
                ss = sb.tile([128, cs, 128], BF16, tag="sc")
                for k in range(cs):
                    nc.vector.scalar_tensor_tensor(
                        out=ss[:, k, :], in0=iota_bf[:],
                        scalar=dst_bf[:, c0 + k:c0 + k + 1],
                        in1=ew_bf[:, c0 + k, None].to_broadcast([128, 128]),
                        op0=OP.is_equal, op1=OP.mult)
                for k in range(cs):
                    last = (s == NSEG - 1 or sum(d.CS_tab[w][s + 1:]) == 0) \
                        and (k == cs - 1)
                    nc.tensor.matmul(pw[:], lhsT=ss[:, k, :],
                                     rhs=yts[s][:, k, :],
                                     start=first, stop=last)
                    first = False
            ywr = sb.tile([128, H], BF16, tag="ywr")
            nc.sync.dma_start(out=ywr[:], in_=y_loc_r[:, w, :])
            t1 = sb.tile([128, H], F32, tag="t1")
            nc.vector.tensor_add(t1[:], pw[:], ywr[:])
            t2 = sb.tile([128, H], F32, tag="t2")
            nc.vector.scalar_tensor_tensor(
                out=t2[:], in0=t1[:], scalar=dinv[:, w:w + 1], in1=bmat_sb[:],
                op0=OP.mult, op1=OP.add)
            x1w = sb.tile([128, H], BF16, tag="x1w")
            nc.scalar.activation(out=x1w[:], in_=t2[:], func=AT.Relu)
            nc.sync.dma_start(out=x1_loc_r[:, w, :], in_=x1w[:])

        x1_full = dr.tile([NCORES * d.NPAD, H], BF16, addr_space="Shared")
        nc.gpsimd.collective_compute(
            "AllGather", OP.bypass, replica_groups=[list(range(NCORES))],
            ins=[x1_loc[:]], outs=[x1_full[:]])

        # ---------- pair decoder ----------
        out_r = out_d.ap().rearrange("a b -> b a")      # [1, PCH2*128]
        for g in range(16):
            PGg = d.PG_tab[g]
            gb = int(d.gbase[g])
            a_seg, b_seg = g // NSEG, g % NSEG
            e1 = gp.tile([128, PGg, H], BF16, tag="et")
            e2 = gp.tile([128, PGg, H], BF16, tag="et")
            for (tile_, src_seg, arr) in ((e1, a_seg, o1g), (e2, b_seg, o2g)):
                ixd = sb.tile([128, PGg * 8], I16, tag="ixd")
                nc.sync.dma_start(out=ixd[:], in_=arr.ap()[:, gb * 8:(gb + PGg) * 8])
                nc.gpsimd.dma_gather(
                    out_ap=tile_[:],
                    in_ap=x1_full[src_seg * d.SEGR:(src_seg + 1) * d.SEGR, :],
                    idxs_ap=ixd[:], num_idxs=PGg * 128,
                    num_idxs_reg=d.PGM[g],
                    elem_size=H, single_packet=False, queue_num=next_q())
            em = gp.tile([128, PGg, H], BF16, tag="et")
            nc.vector.tensor_mul(em[:], e1[:], e2[:])
            NTg = (PGg + 3) // 4
            ob = sb.tile([1, NTg * 512], F32, tag="ob")
            for nt in range(NTg):
                cT = sb.tile([128, 3, 4, 128], BF16, tag="cT")
                for jj in range(min(4, PGg - nt * 4)):
                    j = nt * 4 + jj
                    for cix, src in enumerate((e1, e2, em)):
                        pt = ps.tile([128, 128], BF16, tag="trf")
                        nc.tensor.transpose(pt[:], src[:, j, :], ident[:])
                        if (j + cix) % 2 == 0:
                            nc.scalar.copy(out=cT[:, cix, jj, :], in_=pt[:])
                        else:
                            nc.vector.tensor_copy(out=cT[:, cix, jj, :], in_=pt[:])
                h_sb = sb.tile([128, MB, 512], BF16, tag="hsb")
                for mb in range(MB):
                    ph = ps.tile([128, 512], F32, tag="ph")
                    for cix in range(3):
                        nc.tensor.matmul(
                            ph[:], lhsT=wc_sb[:, cix, mb * 128:(mb + 1) * 128],
                            rhs=cT[:, cix, :, :],
                            start=(cix == 0), stop=(cix == 2))
                    nc.scalar.activation(out=h_sb[:, mb, :], in_=ph[:], func=AT.Relu,
                                         bias=d1b_sb[:, mb:mb + 1])
                po = ps.tile([1, 512], F32, tag="po")
                for mb in range(MB):
                    nc.tensor.matmul(po[:], lhsT=d2r[:, mb, :],
                                     rhs=h_sb[:, mb, :],
                                     start=(mb == 0), stop=(mb == MB - 1))
                nc.scalar.activation(out=ob[:, nt * 512:(nt + 1) * 512], in_=po[:],
                                     func=AT.Identity, bias=d2b_sb[:, :])
            nc.sync.dma_start(out=out_r[:, gb * 128:(gb + PGg) * 128],
                              in_=ob[:, :PGg * 128])

    nc.compile()
    return nc


_CACHE = {}
TRACE = False          # test harness sets True to capture NTFF profile
LAST_RESULT = None     # BassKernelResults of the most recent run


def kernel(**inputs):
    x = np.asarray(inputs["x"], np.float32)
    curvature = np.asarray(inputs["curvature"], np.float32)
    edge_index = np.asarray(inputs["edge_index"])
    idx = np.asarray(inputs["idx"])
    N, FIN = x.shape
    E = edge_index.shape[1]
    P = idx.shape[1]
    H = np.asarray(inputs["W_gcn"]).shape[1]
    D1 = np.asarray(inputs["dec1_w"]).shape[1]

    CS_tab, D, PG_tab, M_tab, PGM = chunk_params(edge_index, idx, N)
    d = Dims(N, E, P, FIN, H, D1, CS_tab, D, PG_tab, M_tab, PGM)
    maps, perms = preprocess(d, x, curvature, edge_index, idx,
                             inputs["b_gcn"], inputs["lin1_w"], inputs["lin1_b"])
    shared = dict(
        w_gcn=np.asarray(inputs["W_gcn"], np.float32),
        dec1_w=np.asarray(inputs["dec1_w"], np.float32),
        dec1_b=np.asarray(inputs["dec1_b"], np.float32).reshape(-1),
        dec2_w=np.asarray(inputs["dec2_w"], np.float32),
        dec2_b=np.asarray(inputs["dec2_b"], np.float32).reshape(-1),
    )
    for m in maps:
        m.update(shared)

    key = (N, E, P, tuple(tuple(r) for r in CS_tab), D, tuple(PG_tab),
           tuple(tuple(r) for r in M_tab), tuple(PGM))
    if key not in _CACHE:
        _CACHE[key] = build(d)
    nc = _CACHE[key]

    from concourse.bass_utils import run_bass_kernel_spmd
    res = run_bass_kernel_spmd(nc, maps, core_ids=list(range(NCORES)),
                               trace=TRACE)
    global LAST_RESULT
    LAST_RESULT = res
    out = np.empty((P, 1), np.float32)
    for c in range(NCORES):
        vals = np.asarray(res.results[c]["out_d"], np.float32)[:, 0]
        pk, slot = perms[c]
        out[c * d.PPC + pk, 0] = vals[slot]
    return out


# revision 50
# speedup vs baseline: 2.0280x; 1.0282x over previous
"""Trainium2 Bass kernel for nn_CGCN (curvature-weighted GCN + pair decoder).

Strategy (8 NeuronCores, SPMD):
  - Edges sharded by DESTINATION node: core c owns nodes [c*N/8, (c+1)*N/8)
    and every edge whose col lands there (host bins/sorts; index plumbing
    only). The scatter-add stays core-local - no [N,H] all-reduce.
  - Edge weights ew = Linear(func_k(curvature)) on device via Horner in
    t = exp(-c).
  - Degrees: per-node padded layout [node-on-partition, window x Dmax],
    fp32 vector reduce (exact per-node sums, no long-prefix cancellation).
  - y = dinv * (x @ W) per 128-node window: x is fed pre-transposed from
    the host so x^T tiles DMA straight into the matmul lhsT (no PE
    transposes); bf16 operands, f32 PSUM. y AllGathered in bf16.
  - Aggregation: y rows fetched with dma_gather (int16 idx; table split in
    4 segments of 2*NPAD rows so indices fit int16). The gather ucode runs
    on one Q7 core pair selected by queue_num, so calls round-robin over 4
    SWDGE queues (num_swdge_queues=4) to overlap descriptor generation -
    the dominant cost at ~9ns/row on a single pair. Chunk counts are
    per-(window, segment), max over the 8 cores (SPMD shares one program);
    idx lists are padded with 0 up to the shared max-count (num_idxs_reg)
    and then -1 to chunk capacity, which the ucode trims from descriptor
    generation (num_idxs_reg must equal the non-negative idx count or the
    decode-side ring reservation leaks and hangs the device). Gather tiles
    are pre-zeroed on the Scalar engine so trimmed (stale) slots can never
    inject NaN into the matmul. Per chunk one fused DVE op builds
    S'[p,j] = (iota_j==dst_rel[p])*ew[p] and PE accumulates
    psum[window,H] += S'^T @ y_rows (edge-weight multiply fused into S').
  - x1 = relu(dinv*(agg+y)+b) in bf16, AllGathered (halves collective and
    decoder-gather bytes vs f32).
  - Decoder: pairs grouped by (seg(e1), seg(e2)) into 16 variable-size
    groups; gathers use the same -1 trim (NaN from stale slots stays
    column-isolated and those columns are discarded by the host
    un-permute). feat@dec1_w decomposed as e1@(Wa+Wc) + e2@(Wa+Wd) +
    (e1*e2)@Wb, all in bf16 (F32R matmuls are ~4x slower on PE).
"""
import sys

for _p in ("/opt/trn_rl_repo",):
    if _p not in sys.path:
        sys.path.append(_p)

import numpy as np
import ml_dtypes
from contextlib import ExitStack

import concourse.bass as bass
import concourse.tile as tile
from concourse import mybir, bacc
from concourse.masks import make_identity

F32 = mybir.dt.float32
F32R = mybir.dt.float32r
BF16 = mybir.dt.bfloat16
I32 = mybir.dt.int32
I16 = mybir.dt.int16

NCORES = 8
NSEG = 4          # table segments (2 cores' rows each) so idx fits int16
WARM_W = 0        # gather tiles are pre-zeroed instead of warmed
PAD_IDX = -1      # -1: ucode trims trailing pads (no Q7 cost); 0: plain pad


def _wrap16(a):
    """[128, n] per-slot idx array -> dma_gather idx layout [128, n*8] int16.

    Slot (chunk c, partition p) maps to idx-list position i = c*128 + p;
    the ucode reads position i from [p16=i%16, col=c*8 + (i%128)//16],
    partitions replicated x8.
    """
    p128, n = a.shape
    assert p128 == 128
    m = np.zeros((16, n, 8), np.int16)
    for s16 in range(8):
        m[:, :, s16] = a[s16 * 16:(s16 + 1) * 16, :]
    m = m.reshape(16, n * 8)
    return np.tile(m, (8, 1)).copy()


class Dims:
    def __init__(self, N, E, P, FIN, H, D1, CS_tab, D, PG_tab, M_tab, PGM):
        self.N, self.E, self.P = N, E, P
        self.FIN, self.H, self.D1 = FIN, H, D1
        self.CS_tab = CS_tab           # [W][NSEG] chunks per (window, seg)
        self.D = D                     # max node in-degree
        self.PG_tab = PG_tab           # [16] decoder chunks per pair-group
        self.M_tab = M_tab             # [W][NSEG] max-over-cores idx count
        self.PGM = PGM                 # [16] max-over-cores pair count
        self.NSH = N // NCORES
        self.W = (self.NSH + 127) // 128
        self.NPAD = self.W * 128
        self.SEGR = 2 * self.NPAD      # rows per table segment
        assert len(CS_tab) == self.W
        # chunk-column base per (w, s); layout is w-major then s
        self.colbase = np.zeros((self.W, NSEG), np.int64)
        acc = 0
        for w in range(self.W):
            for s in range(NSEG):
                self.colbase[w, s] = acc
                acc += CS_tab[w][s]
        self.NCH = acc                 # total chunk columns
        self.WS = [sum(CS_tab[w]) for w in range(self.W)]
        self.CSMAX = max(max(CS_tab[w]) for w in range(self.W))
        self.WD = self.W * D
        self.PPC = P // NCORES
        # decoder groups (chunks, multiple of 4)
        self.gbase = np.zeros(16, np.int64)
        acc = 0
        for g in range(16):
            self.gbase[g] = acc
            acc += PG_tab[g]
        self.PCH2 = acc
        self.KF = FIN // 128
        self.WH = 2 if self.W % 2 == 0 else 1
        assert self.SEGR <= 32768


def chunk_params(edge_index, idx, N):
    """Per-(window,seg) chunk counts / D / decoder group sizes (max over cores)."""
    NSH = N // NCORES
    W = (NSH + 127) // 128
    NPAD = W * 128
    SEGR = 2 * NPAD
    col = np.asarray(edge_index[1], np.int64)
    row = np.asarray(edge_index[0], np.int64)
    order = np.argsort(col, kind="stable")
    col_s, row_s = col[order], row[order]
    D = int(np.bincount(col_s, minlength=N).max())
    seg_of_row = ((row_s // NSH) * NPAD + (row_s % NSH)) // SEGR
    cs_tab = np.zeros((W, NSEG), np.int64)
    m_tab = np.zeros((W, NSEG), np.int64)
    for c in range(NCORES):
        lo, hi = np.searchsorted(col_s, [c * NSH, (c + 1) * NSH])
        wof = (col_s[lo:hi] - c * NSH) >> 7
        sof = seg_of_row[lo:hi]
        cnt = np.bincount(wof * NSEG + sof, minlength=W * NSEG).reshape(W, NSEG)
        cs_tab = np.maximum(cs_tab, (cnt + 127) // 128)
        m_tab = np.maximum(m_tab, cnt)
    cs_tab[:, 0] = np.maximum(cs_tab[:, 0], 1)   # >=1 matmul per window
    CS_tab = [[int(x) for x in cs_tab[w]] for w in range(W)]
    M_tab = [[int(x) for x in m_tab[w]] for w in range(W)]
    # decoder groups
    i1 = np.asarray(idx[0], np.int64)
    i2 = np.asarray(idx[1], np.int64)
    g1 = ((i1 // NSH) * NPAD + (i1 % NSH)) // SEGR
    g2 = ((i2 // NSH) * NPAD + (i2 % NSH)) // SEGR
    PPC = i1.shape[0] // NCORES
    pg = np.zeros(16, np.int64)
    pgm = np.zeros(16, np.int64)
    for c in range(NCORES):
        g = g1[c * PPC:(c + 1) * PPC] * NSEG + g2[c * PPC:(c + 1) * PPC]
        cnt = np.bincount(g, minlength=16)
        pg = np.maximum(pg, (cnt + 127) // 128)
        pgm = np.maximum(pgm, cnt)
    pg = np.maximum(pg, 1)
    PG_tab = [int(x) for x in pg]
    PGM = [int(x) for x in pgm]
    return CS_tab, D, PG_tab, M_tab, PGM


def preprocess(d, x, curvature, edge_index, idx, b_gcn, lin1_w, lin1_b):
    """Index plumbing: sort/bin/pad edges and pairs, build per-core inputs."""
    row = np.asarray(edge_index[0], dtype=np.int64)
    col = np.asarray(edge_index[1], dtype=np.int64)
    cur = np.asarray(curvature, dtype=np.float32)
    order = np.argsort(col, kind="stable")
    row_s, col_s, cur_s = row[order], col[order], cur[order]
    bounds = np.searchsorted(col_s, np.arange(NCORES + 1) * d.NSH)

    iota_f = np.broadcast_to(np.arange(128, dtype=np.float32), (128, 128)).copy()
    b_mat = np.broadcast_to(np.asarray(b_gcn, np.float32)[None, :], (128, d.H)).copy()
    linw_rep = np.broadcast_to(np.asarray(lin1_w, np.float32)[:, 0][None, :], (128, 10)).copy()
    linb_rep = np.full((128, 1), np.float32(np.asarray(lin1_b).reshape(-1)[0]), np.float32)

    def gidx(nodes):
        return ((nodes // d.NSH) * d.NPAD + (nodes % d.NSH)).astype(np.int64)

    maps = []
    perms = []
    for c in range(NCORES):
        lo, hi = bounds[c], bounds[c + 1]
        colr = (col_s[lo:hi] - c * d.NSH).astype(np.int64)
        grow = gidx(row_s[lo:hi])
        curc = cur_s[lo:hi]
        n_c = colr.shape[0]
        wof = colr >> 7
        sof = grow // d.SEGR
        # order edges by (window, segment) [stable within = by col]
        okey = np.argsort(wof * NSEG + sof, kind="stable")
        colr, grow, curc = colr[okey], grow[okey], curc[okey]
        wof, sof = wof[okey], sof[okey]
        # chunk column for each edge
        cnt = np.bincount(wof * NSEG + sof, minlength=d.W * NSEG).reshape(d.W, NSEG)
        start = np.zeros(d.W * NSEG, np.int64)
        start[1:] = np.cumsum(cnt.reshape(-1))[:-1]
        pos = np.arange(n_c) - start[wof * NSEG + sof]
        ci = d.colbase[wof, sof] + (pos >> 7)
        p = (pos & 127).astype(np.int64)

        dstf = np.full((128, d.NCH), -1.0, np.float32)
        dstf[p, ci] = (colr & 127).astype(np.float32)
        curb = np.zeros((128, d.NCH), np.float32)
        curb[p, ci] = curc
        # idx padding: zeros up to the shared per-call count M_tab (so the
        # static num_idxs_reg matches the ucode's trailing-negative trim on
        # every core), then -1 to chunk capacity (trimmed, no Q7 cost).
        ridx = np.full((128, d.NCH), PAD_IDX, np.int64)
        for w in range(d.W):
            for s in range(NSEG):
                cs = d.CS_tab[w][s]
                if cs == 0:
                    continue
                c0 = d.colbase[w, s]
                m = d.M_tab[w][s]
                sl = np.arange(cnt[w, s], m)
                ridx[sl & 127, c0 + (sl >> 7)] = 0
        ridx[p, ci] = grow - sof * d.SEGR
        g16 = _wrap16(ridx.astype(np.int16))

        # degree layout (per-node padded)
        colr2 = col_s[lo:hi] - c * d.NSH
        cur2 = cur_s[lo:hi]
        nstart = np.searchsorted(colr2, np.arange(d.NSH))
        dpos = np.arange(n_c) - nstart[colr2]
        pn = (colr2 & 127).astype(np.int64)
        fi = (colr2 >> 7) * d.D + dpos
        curd = np.zeros((128, d.WD), np.float32)
        curd[pn, fi] = cur2
        maskd = np.zeros((128, d.WD), np.float32)
        maskd[pn, fi] = 1.0

        x_pad = np.zeros((d.NPAD, d.FIN), np.float32)
        x_pad[:d.NSH] = x[c * d.NSH:(c + 1) * d.NSH]
        xt_pad = np.ascontiguousarray(x_pad.T).reshape(d.KF, 128, d.NPAD) \
            .astype(ml_dtypes.bfloat16)

        # decoder pair groups (variable size, 4-chunk padded)
        gi1 = gidx(np.asarray(idx[0][c * d.PPC:(c + 1) * d.PPC], np.int64))
        gi2 = gidx(np.asarray(idx[1][c * d.PPC:(c + 1) * d.PPC], np.int64))
        grp = (gi1 // d.SEGR) * NSEG + (gi2 // d.SEGR)
        pkey = np.argsort(grp, kind="stable")
        gcnt = np.bincount(grp, minlength=16)
        gstart = np.zeros(16, np.int64)
        gstart[1:] = np.cumsum(gcnt)[:-1]
        ppos = np.arange(d.PPC) - gstart[grp[pkey]]
        slot = d.gbase[grp[pkey]] * 128 + ppos      # slot in padded layout
        o1a = np.full((128, d.PCH2), PAD_IDX, np.int64)
        o2a = np.full((128, d.PCH2), PAD_IDX, np.int64)
        for g in range(16):
            sl = np.arange(gcnt[g], d.PGM[g]) + d.gbase[g] * 128
            o1a[sl & 127, sl >> 7] = 0
            o2a[sl & 127, sl >> 7] = 0
        o1a[slot & 127, slot >> 7] = gi1[pkey] % d.SEGR
        o2a[slot & 127, slot >> 7] = gi2[pkey] % d.SEGR
        o1g = _wrap16(o1a.astype(np.int16))
        o2g = _wrap16(o2a.astype(np.int16))
        pk = np.empty(d.PPC, np.int64)
        pk[:] = pkey
        perms.append((pk, np.asarray(slot)))

        maps.append(dict(
            xt_pad=xt_pad, curb=curb, dstf=dstf, g16=g16,
            curd=curd, maskd=maskd, o1g=o1g, o2g=o2g,
            iota_f=iota_f, b_mat=b_mat, linw=linw_rep, linb=linb_rep,
        ))
    return maps, perms


def build(d):
    nc = bacc.Bacc("TRN2", target_bir_lowering=False, debug=False,
                   num_devices=NCORES, num_swdge_queues=4)
    H, D, W = d.H, d.D, d.W
    qrr = [0]          # round-robin SWDGE queue so gathers use all Q7 pairs

    def next_q():
        q = qrr[0] % 4
        qrr[0] += 1
        return q

    xt_pad = nc.dram_tensor("xt_pad", [d.KF, 128, d.NPAD], BF16,
                            kind="ExternalInput")
    curb = nc.dram_tensor("curb", [128, d.NCH], F32, kind="ExternalInput")
    dstf = nc.dram_tensor("dstf", [128, d.NCH], F32, kind="ExternalInput")
    g16 = nc.dram_tensor("g16", [128, d.NCH * 8], I16, kind="ExternalInput")
    curd = nc.dram_tensor("curd", [128, d.WD], F32, kind="ExternalInput")
    maskd = nc.dram_tensor("maskd", [128, d.WD], F32, kind="ExternalInput")
    o1g = nc.dram_tensor("o1g", [128, d.PCH2 * 8], I16, kind="ExternalInput")
    o2g = nc.dram_tensor("o2g", [128, d.PCH2 * 8], I16, kind="ExternalInput")

    iota_f = nc.dram_tensor("iota_f", [128, 128], F32, kind="ExternalInput")
    b_mat = nc.dram_tensor("b_mat", [128, H], F32, kind="ExternalInput")
    linw = nc.dram_tensor("linw", [128, 10], F32, kind="ExternalInput")
    linb = nc.dram_tensor("linb", [128, 1], F32, kind="ExternalInput")
    w_gcn = nc.dram_tensor("w_gcn", [d.FIN, H], F32, kind="ExternalInput")
    dec1_w = nc.dram_tensor("dec1_w", [4 * H, d.D1], F32, kind="ExternalInput")
    dec1_b = nc.dram_tensor("dec1_b", [d.D1], F32, kind="ExternalInput")
    dec2_w = nc.dram_tensor("dec2_w", [d.D1, 1], F32, kind="ExternalInput")
    dec2_b = nc.dram_tensor("dec2_b", [1], F32, kind="ExternalInput")
    out_d = nc.dram_tensor("out_d", [d.PCH2 * 128, 1], F32, kind="ExternalOutput")

    AT = mybir.ActivationFunctionType
    OP = mybir.AluOpType
    MB = d.D1 // 128

    with ExitStack() as ctx:
        tc = ctx.enter_context(tile.TileContext(nc))
        const = ctx.enter_context(tc.tile_pool(name="const", bufs=1))
        sb = ctx.enter_context(tc.tile_pool(name="sb", bufs=3))
        big = ctx.enter_context(tc.tile_pool(name="big", bufs=3))
        gp = ctx.enter_context(tc.tile_pool(name="gp", bufs=5))   # gather tiles
        ps = ctx.enter_context(tc.tile_pool(name="ps", bufs=2, space="PSUM"))
        dr = ctx.enter_context(tc.tile_pool(name="dr", bufs=1, space="DRAM"))

        # ---------- constants ----------
        iota_sb = const.tile([128, 128], F32, tag="iota32")
        nc.sync.dma_start(out=iota_sb[:], in_=iota_f.ap())
        iota_bf = const.tile([128, 128], BF16, tag="iotabf")
        nc.vector.tensor_copy(out=iota_bf[:], in_=iota_sb[:])
        ident = const.tile([128, 128], BF16, tag="ident")
        make_identity(nc, ident[:])
        ident_f = const.tile([128, 128], F32, tag="identf")
        make_identity(nc, ident_f[:])
        bmat_sb = const.tile([128, H], F32, tag="bmat")
        nc.sync.dma_start(out=bmat_sb[:], in_=b_mat.ap())

        linw_sb = const.tile([128, 10], F32, tag="linw")
        nc.sync.dma_start(out=linw_sb[:], in_=linw.ap())
        linb_sb = const.tile([128, 1], F32, tag="linb")
        nc.sync.dma_start(out=linb_sb[:], in_=linb.ap())
        coef = const.tile([128, 10], F32, tag="coef")
        nc.vector.tensor_scalar_mul(coef[:], linw_sb[:], 0.5)
        csum = const.tile([128, 1], F32, tag="csum")
        nc.vector.tensor_reduce(out=csum[:], in_=coef[:], axis=mybir.AxisListType.X,
                                op=OP.add)
        cconst = const.tile([128, 1], F32, tag="cconst")
        nc.vector.tensor_add(cconst[:], csum[:], linb_sb[:])

        wg_sb = const.tile([128, d.KF, H], F32, tag="wg32")
        nc.sync.dma_start(out=wg_sb[:], in_=w_gcn.ap().rearrange("(k p) h -> p k h", p=128))
        wgr = const.tile([128, d.KF, H], BF16, tag="wgr")
        nc.vector.tensor_copy(out=wgr[:], in_=wg_sb[:])

        d1_sb = big.tile([128, 4, d.D1], F32, tag="big")
        nc.sync.dma_start(out=d1_sb[:], in_=dec1_w.ap().rearrange("(b p) d -> p b d", p=128))
        wc_sb = const.tile([128, 3, d.D1], BF16, tag="wcf")
        nc.vector.tensor_add(wc_sb[:, 0, :], d1_sb[:, 0, :], d1_sb[:, 2, :])
        nc.vector.tensor_add(wc_sb[:, 1, :], d1_sb[:, 0, :], d1_sb[:, 3, :])
        nc.vector.tensor_copy(out=wc_sb[:, 2, :], in_=d1_sb[:, 1, :])

        d1b_sb = const.tile([128, MB], F32, tag="d1b")
        nc.sync.dma_start(out=d1b_sb[:], in_=dec1_b.ap().rearrange("(b p) -> p b", p=128))
        d2_sb = const.tile([128, MB, 1], F32, tag="d232")
        nc.sync.dma_start(out=d2_sb[:], in_=dec2_w.ap().rearrange("(b p) o -> p b o", p=128))
        d2r = const.tile([128, MB, 1], BF16, tag="d2r")
        nc.vector.tensor_copy(out=d2r[:], in_=d2_sb[:])
        d2b_sb = const.tile([1, 1], F32, tag="d2b")
        nc.sync.dma_start(out=d2b_sb[:], in_=dec2_b.ap()[:, None])

        zero_c = const.tile([128, d.CSMAX, H], BF16, tag="zeroc")
        nc.vector.memset(zero_c[:], 0.0)

        # ---------- edge weights (chunk layout) ----------
        def horner(src_ap, n):
            t = big.tile([128, n], F32, tag="big")
            nc.scalar.activation(out=t[:], in_=src_ap, func=AT.Exp, scale=-1.0)
            acc = big.tile([128, n], F32, tag="big")
            nc.vector.tensor_scalar_mul(acc[:], t[:], coef[:, 9:10])
            for k in range(8, -1, -1):
                nc.vector.scalar_tensor_tensor(
                    out=acc[:], in0=acc[:], scalar=coef[:, k:k + 1], in1=t[:],
                    op0=OP.add, op1=OP.mult)
            nc.vector.tensor_scalar_add(acc[:], acc[:], cconst[:])
            return acc

        curb_sb = big.tile([128, d.NCH], F32, tag="big")
        nc.sync.dma_start(out=curb_sb[:], in_=curb.ap())
        ew_nch = horner(curb_sb[:], d.NCH)
        ew_bf = const.tile([128, d.NCH], BF16, tag="ewf")
        nc.vector.tensor_copy(out=ew_bf[:], in_=ew_nch[:])

        dst_sb = big.tile([128, d.NCH], F32, tag="big")
        nc.sync.dma_start(out=dst_sb[:], in_=dstf.ap())
        dst_bf = const.tile([128, d.NCH], BF16, tag="dstbf")
        nc.vector.tensor_copy(out=dst_bf[:], in_=dst_sb[:])

        # ---------- degrees ----------
        WHF = d.WD // d.WH
        WHW = W // d.WH
        deg_r = const.tile([128, W], F32, tag="deg")
        for h in range(d.WH):
            cu = big.tile([128, WHF], F32, tag="big")
            nc.sync.dma_start(out=cu[:], in_=curd.ap()[:, h * WHF:(h + 1) * WHF])
            ewd = horner(cu[:], WHF)
            mk = big.tile([128, WHF], F32, tag="big")
            nc.sync.dma_start(out=mk[:], in_=maskd.ap()[:, h * WHF:(h + 1) * WHF])
            nc.vector.tensor_mul(ewd[:], ewd[:], mk[:])
            nc.vector.tensor_reduce(
                out=deg_r[:, h * WHW:(h + 1) * WHW],
                in_=ewd[:].rearrange("p (w dd) -> p w dd", dd=D),
                axis=mybir.AxisListType.X, op=OP.add)
        nc.vector.tensor_scalar_add(deg_r[:], deg_r[:], 1.0)
        mw = const.tile([128, W], F32, tag="mw")
        nc.vector.tensor_single_scalar(out=mw[:], in_=deg_r[:], scalar=0.0, op=OP.is_gt)
        degm = const.tile([128, W], F32, tag="degm")
        nc.vector.tensor_mul(degm[:], deg_r[:], mw[:])
        onem = const.tile([128, W], F32, tag="onem")
        nc.vector.tensor_scalar(out=onem[:], in0=mw[:], scalar1=-1.0, scalar2=1.0,
                                op0=OP.mult, op1=OP.add)
        nc.vector.tensor_add(degm[:], degm[:], onem[:])
        rec = const.tile([128, W], F32, tag="rec")
        nc.vector.reciprocal(out=rec[:], in_=degm[:])
        dsq = const.tile([128, W], F32, tag="dsq")
        nc.scalar.activation(out=dsq[:], in_=rec[:], func=AT.Sqrt)
        dinv = const.tile([128, W], F32, tag="dinv")
        nc.vector.tensor_mul(dinv[:], dsq[:], mw[:])

        # ---------- xw + y per window ----------
        y_loc = dr.tile([d.NPAD, H], BF16)
        y_loc_r = y_loc[:].rearrange("(w p) h -> p w h", p=128)
        for w in range(W):
            xt = sb.tile([128, d.KF, 128], BF16, tag="xt")
            nc.sync.dma_start(
                out=xt[:],
                in_=xt_pad.ap().rearrange("k p n -> p k n")[:, :, w * 128:(w + 1) * 128])
            pxw = ps.tile([128, H], F32, tag="win")
            for k in range(d.KF):
                nc.tensor.matmul(pxw[:], lhsT=xt[:, k, :], rhs=wgr[:, k, :],
                                 start=(k == 0), stop=(k == d.KF - 1))
            yw = sb.tile([128, H], BF16, tag="yw")
            nc.scalar.activation(out=yw[:], in_=pxw[:], func=AT.Copy,
                                 scale=dinv[:, w:w + 1])
            nc.sync.dma_start(out=y_loc_r[:, w, :], in_=yw[:])

        y_full = dr.tile([NCORES * d.NPAD, H], BF16, addr_space="Shared")
        nc.gpsimd.collective_compute(
            "AllGather", OP.bypass, replica_groups=[list(range(NCORES))],
            ins=[y_loc[:]], outs=[y_full[:]])

        # ---------- aggregation ----------
        x1_loc = dr.tile([d.NPAD, H], BF16)
        x1_loc_r = x1_loc[:].rearrange("(w p) h -> p w h", p=128)
        for w in range(W):
            cb = d.colbase[w]
            # one idx load covering this window's 4 segment calls
            ix = sb.tile([128, d.WS[w] * 8], I16, tag="ix")
            nc.sync.dma_start(out=ix[:], in_=g16.ap()[:, cb[0] * 8:(cb[0] + d.WS[w]) * 8])
            yts = []
            for s in range(NSEG):
                cs = d.CS_tab[w][s]
                if cs == 0:
                    yts.append(None)
                    continue
                off = cb[s] - cb[0]
                yt = gp.tile([128, cs, H], BF16, tag="yt")
                nc.scalar.copy(out=yt[:], in_=zero_c[:, :cs, :])
                nc.gpsimd.dma_gather(
                    out_ap=yt[:], in_ap=y_full[s * d.SEGR:(s + 1) * d.SEGR, :],
                    idxs_ap=ix[:, off * 8:(off + cs) * 8],
                    num_idxs=cs * 128, num_idxs_reg=d.M_tab[w][s],
                    elem_size=H, single_packet=False, queue_num=next_q())
                yts.append(yt)
            pw = ps.tile([128, H], F32, tag="win")
            first = True
            for s in range(NSEG):
                cs = d.CS_tab[w][s]
                if cs == 0:
                    continue
                c0 = cb[s]
                # S'[p,k,j] = (iota_j == dst[p,k]) * ew[p,k]; then
                # pw += S'^T @ y_rows fuses the edge-weight multiply into PE.
                ss = sb.tile([128, cs, 128], BF16, tag="sc")
                for k in range(cs):
                    nc.vector.scalar_tensor_tensor(
                        out=ss[:, k, :], in0=iota_bf[:],
                        scalar=dst_bf[:, c0 + k:c0 + k + 1],
                        in1=ew_bf[:, c0 + k, None].to_broadcast([128, 128]),
                        op0=OP.is_equal, op1=OP.mult)
                for k in range(cs):
                    last = (s == NSEG - 1 or sum(d.CS_tab[w][s + 1:]) == 0) \
                        and (k == cs - 1)
                    nc.tensor.matmul(pw[:], lhsT=ss[:, k, :],
                                     rhs=yts[s][:, k, :],
                                     start=first, stop=last)
                    first = False
            ywr = sb.tile([128, H], BF16, tag="ywr")
            nc.sync.dma_start(out=ywr[:], in_=y_loc_r[:, w, :])
            t1 = sb.tile([128, H], F32, tag="t1")
            nc.vector.tensor_add(t1[:], pw[:], ywr[:])
            t2 = sb.tile([128, H], F32, tag="t2")
            nc.vector.scalar_tensor_tensor(
                out=t2[:], in0=t1[:], scalar=dinv[:, w:w + 1], in1=bmat_sb[:],
                op0=OP.mult, op1=OP.add)
            x1w = sb.tile([128, H], BF16, tag="x1w")
            nc.scalar.activation(out=x1w[:], in_=t2[:], func=AT.Relu)
            nc.sync.dma_start(out=x1_loc_r[:, w, :], in_=x1w[:])

        x1_full = dr.tile([NCORES * d.NPAD, H], BF16, addr_space="Shared")
        nc.gpsimd.collective_compute(
            "AllGather", OP.bypass, replica_groups=[list(range(NCORES))],
            ins=[x1_loc[:]], outs=[x1_full[:]])

        # ---------- pair decoder ----------
        # transpose-mode gathers return x1 rows as columns [H, pairs] - the
        # exact rhs layout feat^T the dec1 matmul needs; no PE transposes.
        out_r = out_d.ap().rearrange("a b -> b a")      # [1, PCH2*128]
        for g in range(16):
            PGg = d.PG_tab[g]
            gb = int(d.gbase[g])
            a_seg, b_seg = g // NSEG, g % NSEG
            e1 = gp.tile([128, 1, PGg * 128], BF16, tag="et")
            e2 = gp.tile([128, 1, PGg * 128], BF16, tag="et")
            for (tile_, src_seg, arr) in ((e1, a_seg, o1g), (e2, b_seg, o2g)):
                ixd = sb.tile([128, PGg * 8], I16, tag="ixd")
                nc.sync.dma_start(out=ixd[:], in_=arr.ap()[:, gb * 8:(gb + PGg) * 8])
                nc.gpsimd.dma_gather(
                    out_ap=tile_[:],
                    in_ap=x1_full[src_seg * d.SEGR:(src_seg + 1) * d.SEGR, :],
                    idxs_ap=ixd[:], num_idxs=PGg * 128,
                    num_idxs_reg=d.PGM[g], transpose=True,
                    elem_size=H, single_packet=False, queue_num=next_q())
            em = gp.tile([128, 1, PGg * 128], BF16, tag="et")
            nc.vector.tensor_mul(em[:], e1[:], e2[:])
            NTg = (PGg + 3) // 4
            ob = sb.tile([1, NTg * 512], F32, tag="ob")
            for nt in range(NTg):
                npair = min(512, PGg * 128 - nt * 512)
                h_sb = sb.tile([128, MB, 512], BF16, tag="hsb")
                for mb in range(MB):
                    ph = ps.tile([128, 512], F32, tag="ph")
                    for cix, src in enumerate((e1, e2, em)):
                        nc.tensor.matmul(
                            ph[:, :npair],
                            lhsT=wc_sb[:, cix, mb * 128:(mb + 1) * 128],
                            rhs=src[:, 0, nt * 512:nt * 512 + npair],
                            start=(cix == 0), stop=(cix == 2))
                    nc.scalar.activation(out=h_sb[:, mb, :npair], in_=ph[:, :npair],
                                         func=AT.Relu, bias=d1b_sb[:, mb:mb + 1])
                po = ps.tile([1, 512], F32, tag="po")
                for mb in range(MB):
                    nc.tensor.matmul(po[:], lhsT=d2r[:, mb, :],
                                     rhs=h_sb[:, mb, :],
                                     start=(mb == 0), stop=(mb == MB - 1))
                nc.scalar.activation(out=ob[:, nt * 512:(nt + 1) * 512], in_=po[:],
                                     func=AT.Identity, bias=d2b_sb[:, :])
            nc.sync.dma_start(out=out_r[:, gb * 128:(gb + PGg) * 128],
                              in_=ob[:, :PGg * 128])

    nc.compile()
    return nc


_CACHE = {}
TRACE = False          # test harness sets True to capture NTFF profile
LAST_RESULT = None     # BassKernelResults of the most recent run


def kernel(**inputs):
    x = np.asarray(inputs["x"], np.float32)
    curvature = np.asarray(inputs["curvature"], np.float32)
    edge_index = np.asarray(inputs["edge_index"])
    idx = np.asarray(inputs["idx"])
    N, FIN = x.shape
    E = edge_index.shape[1]
    P = idx.shape[1]
    H = np.asarray(inputs["W_gcn"]).shape[1]
    D1 = np.asarray(inputs["dec1_w"]).shape[1]

    CS_tab, D, PG_tab, M_tab, PGM = chunk_params(edge_index, idx, N)
    d = Dims(N, E, P, FIN, H, D1, CS_tab, D, PG_tab, M_tab, PGM)
    maps, perms = preprocess(d, x, curvature, edge_index, idx,
                             inputs["b_gcn"], inputs["lin1_w"], inputs["lin1_b"])
    shared = dict(
        w_gcn=np.asarray(inputs["W_gcn"], np.float32),
        dec1_w=np.asarray(inputs["dec1_w"], np.float32),
        dec1_b=np.asarray(inputs["dec1_b"], np.float32).reshape(-1),
        dec2_w=np.asarray(inputs["dec2_w"], np.float32),
        dec2_b=np.asarray(inputs["dec2_b"], np.float32).reshape(-1),
    )
    for m in maps:
        m.update(shared)

    key = (N, E, P, tuple(tuple(r) for r in CS_tab), D, tuple(PG_tab),
           tuple(tuple(r) for r in M_tab), tuple(PGM))
    if key not in _CACHE:
        _CACHE[key] = build(d)
    nc = _CACHE[key]

    from concourse.bass_utils import run_bass_kernel_spmd
    res = run_bass_kernel_spmd(nc, maps, core_ids=list(range(NCORES)),
                               trace=TRACE)
    global LAST_RESULT
    LAST_RESULT = res
    out = np.empty((P, 1), np.float32)
    for c in range(NCORES):
        vals = np.asarray(res.results[c]["out_d"], np.float32)[:, 0]
        pk, slot = perms[c]
        out[c * d.PPC + pk, 0] = vals[slot]
    return out
